# revision 8
# baseline (speedup 1.0000x reference)
"""GCN (4-layer message-passing) Trainium2 kernel, 8-core SPMD.

Math (matches PyG GCNConv with self-loops, per reference):
    deg[d]  = in-degree over (edges + self-loops)
    dinv    = deg^-1/2
    h0      = x @ W_emb + b_emb
    layer i: h <- tanh( dinv[d] * sum_{e: dst=d} dinv[src_e] * (h @ W_i)[src_e] + b_i )
    out     = h @ W_out + b_out

Distribution: nodes sharded across 8 cores (dst-sharded edges). Per layer:
  1. transform own shard:  hWd = dinv * (h @ W)   (PE matmul + ACT scale/cast bf16)
  2. AllGather hWd across cores (bf16, HBM collective)
  3. dma_gather (SWDGE) each in-edge's source row, sorted by dst
  4. segment-sum via PE matmuls against host-built one-hot selection tiles
     (exact in bf16), accumulated in PSUM; bias added via rank-1 matmul
  5. tanh with per-partition dinv scale on ACT; PE-transpose back to h^T

The embedding layer is folded into layer 1's weights host-side
(x @ (W_emb@W_1) + (b_emb@W_1)), so it never runs on device.

int16 gather indices cap the addressable rows at 32768, so the AllGather'd
feature table is gathered through two views (rows [0,32768) and [32768,...)),
with each block's edges grouped lo-half-first.
"""

import math

import ml_dtypes
import numpy as np

BF16 = ml_dtypes.bfloat16
P = 128

CFG_FULL = dict(N=50000, E=800000, DIN=128, DH=256, DOUT=64, L=4, NC=8)

# blocks of 128 dst nodes handled per gather/S-matrix chunk
CHUNK_BLOCKS = 2


def kernel(**inputs) -> np.ndarray:
    out, _ = run(inputs, CFG_FULL)
    return out


# ----------------------------------------------------------------------------
# host-side preprocessing
# ----------------------------------------------------------------------------


def _ceil_div(a, b):
    return (a + b - 1) // b


def preprocess(inputs, cfg):
    N, E, DIN, DH, DOUT, L, NC = (
        cfg["N"], cfg["E"], cfg["DIN"], cfg["DH"], cfg["DOUT"], cfg["L"], cfg["NC"],
    )
    x = np.asarray(inputs["x"], np.float32)
    ei = np.asarray(inputs["edge_index"]).astype(np.int64)
    W_emb = np.asarray(inputs["W_emb"], np.float32)
    b_emb = np.asarray(inputs["b_emb"], np.float32)
    W_conv = np.asarray(inputs["W_conv"], np.float32)
    b_conv = np.asarray(inputs["b_conv"], np.float32)
    W_out = np.asarray(inputs["W_out"], np.float32)
    b_out = np.asarray(inputs["b_out"], np.float32)

    loop = np.arange(N, dtype=np.int64)
    src = np.concatenate([ei[0], loop])
    dst = np.concatenate([ei[1], loop])
    deg = np.bincount(dst, minlength=N).astype(np.float32)
    dinv = (1.0 / np.sqrt(np.maximum(deg, 1.0))).astype(np.float32)
    sqdeg = np.sqrt(np.maximum(deg, 1.0)).astype(np.float32)

    NPs = _ceil_div(N, NC)          # real nodes per shard
    NB = _ceil_div(NPs, P)          # dst blocks per core
    NPP = NB * P                    # padded nodes per shard
    AGR = NC * NPP                  # allgather rows
    HALF = 32768
    has_hi = AGR > HALF

    agrow = (src // NPs) * NPP + (src % NPs)
    core_of = dst // NPs
    d_loc = dst - core_of * NPs
    blk = d_loc // P
    col = d_loc % P
    half = (agrow >= HALF).astype(np.int64)

    # per-core edge partitions, sorted by (block, half, dst, src)
    cores = []
    nseg = np.zeros((NC, NB, 2), np.int64)
    for c in range(NC):
        m = core_of == c
        a_blk, a_half, a_col, a_sag = blk[m], half[m], col[m], agrow[m]
        order = np.lexsort((a_sag, a_col, a_half, a_blk))
        a_blk, a_half, a_col, a_sag = (
            a_blk[order], a_half[order], a_col[order], a_sag[order],
        )
        cnt = np.bincount(a_blk * 2 + a_half, minlength=NB * 2).reshape(NB, 2)
        nseg[c] = cnt
        cores.append((a_blk, a_half, a_col, a_sag))

    nmax = nseg.max(axis=0)                      # [NB, 2]
    T = np.maximum(_ceil_div(nmax, P), 0)        # tiles per (block, half)
    T[:, 0] = np.maximum(T[:, 0], 1)             # every block has >= 1 lo tile slot
    if not has_hi:
        T[:, 1] = 0

    # chunk layout (identical across cores)
    chunks = []
    gidx_col = 0
    tile_ctr = 0
    for g0 in range(0, NB, CHUNK_BLOCKS):
        blocks = list(range(g0, min(g0 + CHUNK_BLOCKS, NB)))
        tlo = int(T[blocks, 0].sum())
        thi = int(T[blocks, 1].sum())
        lo_cols = (gidx_col, gidx_col + tlo * P // 16)
        gidx_col = lo_cols[1]
        hi_cols = (gidx_col, gidx_col + thi * P // 16)
        gidx_col = hi_cols[1]
        # tile positions: lo tiles of blocks in order, then hi tiles
        lo_base, hi_base = {}, {}
        t = 0
        for b in blocks:
            lo_base[b] = t
            t += int(T[b, 0])
        t = 0
        for b in blocks:
            hi_base[b] = t
            t += int(T[b, 1])
        smat_tiles = (tile_ctr, tile_ctr + tlo + thi)
        tile_ctr = smat_tiles[1]
        chunks.append(dict(
            blocks=blocks, tlo=tlo, thi=thi, lo_cols=lo_cols, hi_cols=hi_cols,
            lo_base=lo_base, hi_base=hi_base, smat_tiles=smat_tiles,
        ))
    GC = gidx_col
    TT = tile_ctr

    meta = dict(
        NPs=NPs, NB=NB, NPP=NPP, AGR=AGR, HALF=HALF, has_hi=has_hi,
        T=T, chunks=chunks, GC=GC, TT=TT,
    )

    # shared weights
    W1x = (W_emb @ W_conv[0]).astype(BF16)                    # [DIN, DH]
    bemb1 = (b_emb @ W_conv[0]).reshape(1, DH).astype(BF16)
    Wc = W_conv[1:].reshape((L - 1) * DH, DH).astype(BF16) if L > 1 else \
        np.zeros((0, DH), BF16)
    bc = b_conv.reshape(1, L * DH).astype(BF16)
    Wo = W_out.astype(BF16)                                    # [DH, DOUT]
    bo = b_out.reshape(1, DOUT).astype(BF16)

    in_maps = []
    for c in range(NC):
        a_blk, a_half, a_col, a_sag = cores[c]
        n0 = c * NPs
        n1 = min(n0 + NPs, N)
        nreal = n1 - n0

        # per-edge destination slots in the (chunk-ordered) tile stream
        # segment offsets within each (blk, half) run
        seg_id = a_blk * 2 + a_half
        seg_start = np.zeros(NB * 2, np.int64)
        cnts = np.bincount(seg_id, minlength=NB * 2)
        seg_start[1:] = np.cumsum(cnts)[:-1]
        epos = np.arange(len(seg_id)) - seg_start[seg_id]     # offset in segment

        # global tile index for each edge
        tile_of_seg = np.zeros(NB * 2, np.int64)
        for ch in chunks:
            for b in ch["blocks"]:
                tile_of_seg[b * 2] = ch["smat_tiles"][0] + ch["lo_base"][b]
                tile_of_seg[b * 2 + 1] = (
                    ch["smat_tiles"][0] + ch["tlo"] + ch["hi_base"][b]
                )
        e_tile = tile_of_seg[seg_id] + epos // P
        e_row = epos % P

        # selection matrices [128, TT*128] bf16, one-hot (exact)
        smat = np.zeros((P, TT * P), BF16)
        smat[e_row, e_tile * P + a_col] = BF16(1.0)

        # gather indices, wrapped layout [128, GC] int16
        gidx = np.zeros((16, GC), np.int16)
        for ch in chunks:
            for h, colrange, base_map, tcount in (
                (0, ch["lo_cols"], ch["lo_base"], ch["tlo"]),
                (1, ch["hi_cols"], ch["hi_base"], ch["thi"]),
            ):
                if tcount == 0:
                    continue
                vals = np.zeros(tcount * P, np.int64)
                for b in ch["blocks"]:
                    m = (a_blk == b) & (a_half == h)
                    v = a_sag[m] - (HALF if h else 0)
                    off = base_map[b] * P
                    vals[off:off + len(v)] = v
                c0, c1 = colrange
                gidx[:, c0:c1] = vals.reshape(c1 - c0, 16).T
        gidx = np.tile(gidx, (8, 1)).astype(np.int16)          # replicate x8

        # dinv [128, NB] fp32 ; sqdeg [1, NPP] bf16
        dl = np.ones(NPP, np.float32)
        dl[:nreal] = dinv[n0:n1]
        dinvp = dl.reshape(NB, P).T.copy()                     # [P, NB]
        sq = np.ones(NPP, np.float32)
        sq[:nreal] = sqdeg[n0:n1]
        sqdegp = sq.reshape(1, NPP).astype(BF16)

        xT = np.zeros((DIN, NPP), BF16)
        xT[:, :nreal] = x[n0:n1].T

        in_maps.append(dict(
            xT=np.ascontiguousarray(xT),
            gidx=np.ascontiguousarray(gidx),
            smat=np.ascontiguousarray(smat),
            dinvp=np.ascontiguousarray(dinvp),
            sqdegp=np.ascontiguousarray(sqdegp),
            w1x=W1x, bemb1=bemb1, wc=Wc, bc=bc, wo=Wo, bo=bo,
        ))

    return in_maps, meta


# ----------------------------------------------------------------------------
# device program
# ----------------------------------------------------------------------------


def build_program(meta, cfg):
    import concourse.bacc as bacc
    import concourse.mybir as mybir
    import concourse.tile as tile
    from concourse.masks import make_identity

    N, DIN, DH, DOUT, L, NC = (
        cfg["N"], cfg["DIN"], cfg["DH"], cfg["DOUT"], cfg["L"], cfg["NC"],
    )
    NPs, NB, NPP, AGR, HALF = (
        meta["NPs"], meta["NB"], meta["NPP"], meta["AGR"], meta["HALF"],
    )
    T, chunks, GC, TT = meta["T"], meta["chunks"], meta["GC"], meta["TT"]
    has_hi = meta["has_hi"]
    assert DIN == P

    f32 = mybir.dt.float32
    bf16 = mybir.dt.bfloat16
    i16 = mybir.dt.int16
    TANH = mybir.ActivationFunctionType.Tanh
    COPY = mybir.ActivationFunctionType.Copy

    nc = bacc.Bacc("TRN2", target_bir_lowering=False, debug=False, num_devices=NC)

    # I/O
    d_xT = nc.dram_tensor("xT", [DIN, NPP], bf16, kind="ExternalInput")
    d_gidx = nc.dram_tensor("gidx", [P, GC], i16, kind="ExternalInput")
    d_smat = nc.dram_tensor("smat", [P, TT * P], bf16, kind="ExternalInput")
    d_dinv = nc.dram_tensor("dinvp", [P, NB], f32, kind="ExternalInput")
    d_sqdeg = nc.dram_tensor("sqdegp", [1, NPP], bf16, kind="ExternalInput")
    d_w1x = nc.dram_tensor("w1x", [DIN, DH], bf16, kind="ExternalInput")
    d_bemb1 = nc.dram_tensor("bemb1", [1, DH], bf16, kind="ExternalInput")
    d_wc = nc.dram_tensor("wc", [(L - 1) * DH, DH], bf16, kind="ExternalInput")
    d_bc = nc.dram_tensor("bc", [1, L * DH], bf16, kind="ExternalInput")
    d_wo = nc.dram_tensor("wo", [DH, DOUT], bf16, kind="ExternalInput")
    d_bo = nc.dram_tensor("bo", [1, DOUT], bf16, kind="ExternalInput")
    d_out = nc.dram_tensor("out", [NPs, DOUT], f32, kind="ExternalOutput")

    with tile.TileContext(nc) as tc:
        pers = tc.alloc_tile_pool(name="pers", bufs=1)
        dpool = tc.alloc_tile_pool(name="dpers", bufs=1, space="DRAM")

        # internal DRAM for the collective
        agin = dpool.tile([NPP, DH], bf16, name="agin", tag="agin")
        agouts = [
            dpool.tile([AGR, DH], bf16, name=f"agout{i}", tag=f"agout{i}",
                       addr_space="Shared")
            for i in range(L)
        ]

        def stile(shape, dtype, name):
            return pers.tile(shape, dtype, name=name, tag=name)

        # persistent SBUF state
        xT = stile([DIN, NPP], bf16, "xT_sb")
        hT0 = stile([P, NPP], bf16, "hT0_sb")
        hT1 = stile([P, NPP], bf16, "hT1_sb")
        gidx = stile([P, GC], i16, "gidx_sb")
        dinv = stile([P, NB], f32, "dinv_sb")
        sqd = stile([1, NPP], bf16, "sqdeg_sb")
        w1x = stile([DIN, DH], bf16, "w1x_sb")
        bemb1 = stile([1, DH], bf16, "bemb1_sb")
        wc = stile([P, 2 * (L - 1) * DH], bf16, "wc_sb")
        bc = stile([1, L * DH], bf16, "bc_sb")
        wo = stile([P, 2 * DOUT], bf16, "wo_sb")
        bo = stile([1, DOUT], bf16, "bo_sb")
        ident = stile([P, P], bf16, "ident_sb")
        ones = stile([1, P], bf16, "ones_sb")

        nc.sync.dma_start(out=xT[:], in_=d_xT[:])
        nc.sync.dma_start(out=gidx[:], in_=d_gidx[:])
        nc.sync.dma_start(out=dinv[:], in_=d_dinv[:])
        nc.sync.dma_start(out=sqd[:], in_=d_sqdeg[:])
        nc.sync.dma_start(out=w1x[:], in_=d_w1x[:])
        nc.sync.dma_start(out=bemb1[:], in_=d_bemb1[:])
        for i in range(L - 1):
            for k in range(2):
                nc.sync.dma_start(
                    out=wc[:, (2 * i + k) * DH:(2 * i + k + 1) * DH],
                    in_=d_wc[i * DH + k * P:i * DH + (k + 1) * P, :],
                )
        nc.sync.dma_start(out=bc[:], in_=d_bc[:])
        for k in range(2):
            nc.sync.dma_start(
                out=wo[:, k * DOUT:(k + 1) * DOUT],
                in_=d_wo[k * P:(k + 1) * P, :],
            )
        nc.sync.dma_start(out=bo[:], in_=d_bo[:])
        make_identity(nc, ident[:])
        nc.gpsimd.memset(ones[:], 1.0)

        with tc.tile_pool(name="work", bufs=2) as wp, \
                tc.tile_pool(name="psum", bufs=2, space="PSUM") as pp:

            for i in range(L):
                # ---- transform: hWd = dinv * (h @ W_i), cast bf16 -> agin
                for b in range(NB):
                    pt = pp.tile([P, DH], f32, tag="pt")
                    bs = slice(b * P, (b + 1) * P)
                    if i == 0:
                        nc.tensor.matmul(
                            out=pt[:], lhsT=xT[:, bs], rhs=w1x[:],
                            start=True, stop=False,
                        )
                        nc.tensor.matmul(
                            out=pt[:], lhsT=ones[:, :], rhs=bemb1[:],
                            start=False, stop=True,
                        )
                    else:
                        j = i - 1
                        nc.tensor.matmul(
                            out=pt[:], lhsT=hT0[:, bs],
                            rhs=wc[:, (2 * j) * DH:(2 * j + 1) * DH],
                            start=True, stop=False,
                        )
                        nc.tensor.matmul(
                            out=pt[:], lhsT=hT1[:, bs],
                            rhs=wc[:, (2 * j + 1) * DH:(2 * j + 2) * DH],
                            start=False, stop=True,
                        )
                    hwd = wp.tile([P, DH], bf16, tag="hwd")
                    nc.scalar.activation(
                        out=hwd[:], in_=pt[:], func=COPY,
                        scale=dinv[:, b:b + 1],
                    )
                    nc.sync.dma_start(out=agin[bs, :], in_=hwd[:])

                # ---- allgather transformed features
                nc.gpsimd.collective_compute(
                    "AllGather",
                    mybir.AluOpType.bypass,
                    replica_groups=[list(range(NC))],
                    ins=[agin[:]],
                    outs=[agouts[i][:]],
                )

                # ---- gather + segment-sum + tanh + transpose
                for ch in chunks:
                    tlo, thi = ch["tlo"], ch["thi"]
                    msg_lo = wp.tile([P, tlo * DH], bf16, tag="msglo")
                    nc.gpsimd.dma_gather(
                        out_ap=msg_lo[:].rearrange("p (t e) -> p t e", e=DH),
                        in_ap=agouts[i][:min(HALF, AGR), :],
                        idxs_ap=gidx[:, ch["lo_cols"][0]:ch["lo_cols"][1]],
                        num_idxs=tlo * P,
                        num_idxs_reg=tlo * P,
                        elem_size=DH,
                        single_packet=False,
                    )
                    if thi > 0:
                        msg_hi = wp.tile([P, thi * DH], bf16, tag="msghi")
                        nc.gpsimd.dma_gather(
                            out_ap=msg_hi[:].rearrange("p (t e) -> p t e", e=DH),
                            in_ap=agouts[i][HALF:AGR, :],
                            idxs_ap=gidx[:, ch["hi_cols"][0]:ch["hi_cols"][1]],
                            num_idxs=thi * P,
                            num_idxs_reg=thi * P,
                            elem_size=DH,
                            single_packet=False,
                        )
                    smt = wp.tile([P, (tlo + thi) * P], bf16, tag="smat")
                    t0 = ch["smat_tiles"][0]
                    nc.sync.dma_start(
                        out=smt[:], in_=d_smat[:, t0 * P:(t0 + tlo + thi) * P],
                    )

                    for b in ch["blocks"]:
                        pa = pp.tile([P, DH], f32, tag="pa")
                        nmm = int(T[b, 0]) + int(T[b, 1])
                        j = 0
                        for t in range(int(T[b, 0])):
                            s_pos = ch["lo_base"][b] + t
                            nc.tensor.matmul(
                                out=pa[:],
                                lhsT=smt[:, s_pos * P:(s_pos + 1) * P],
                                rhs=msg_lo[:, s_pos * DH:(s_pos + 1) * DH],
                                start=(j == 0), stop=False,
                            )
                            j += 1
                        for t in range(int(T[b, 1])):
                            s_pos = ch["hi_base"][b] + t
                            nc.tensor.matmul(
                                out=pa[:],
                                lhsT=smt[:, (tlo + s_pos) * P:(tlo + s_pos + 1) * P],
                                rhs=msg_hi[:, s_pos * DH:(s_pos + 1) * DH],
                                start=(j == 0), stop=False,
                            )
                            j += 1
                        assert j == nmm
                        bs = slice(b * P, (b + 1) * P)
                        # + sqdeg[d] * b_i  (cancels the dinv scale below)
                        nc.tensor.matmul(
                            out=pa[:], lhsT=sqd[:, bs],
                            rhs=bc[:, i * DH:(i + 1) * DH],
                            start=False, stop=True,
                        )
                        hnew = wp.tile([P, DH], bf16, tag="hnew")
                        nc.scalar.activation(
                            out=hnew[:], in_=pa[:], func=TANH,
                            scale=dinv[:, b:b + 1],
                        )
                        for k, hT in enumerate((hT0, hT1)):
                            ptr = pp.tile([P, P], bf16, tag="ptr")
                            nc.tensor.transpose(
                                out=ptr[:], in_=hnew[:, k * P:(k + 1) * P],
                                identity=ident[:],
                            )
                            nc.vector.tensor_copy(out=hT[:, bs], in_=ptr[:])

            # ---- output projection
            for b in range(NB):
                po = pp.tile([P, DOUT], f32, tag="pt")
                bs = slice(b * P, (b + 1) * P)
                nc.tensor.matmul(out=po[:], lhsT=hT0[:, bs], rhs=wo[:, :DOUT],
                                 start=True, stop=False)
                nc.tensor.matmul(out=po[:], lhsT=hT1[:, bs],
                                 rhs=wo[:, DOUT:2 * DOUT],
                                 start=False, stop=False)
                nc.tensor.matmul(out=po[:], lhsT=ones[:, :], rhs=bo[:],
                                 start=False, stop=True)
                osb = wp.tile([P, DOUT], f32, tag="osb")
                nc.scalar.activation(out=osb[:], in_=po[:], func=COPY)
                rows = min(P, NPs - b * P)
                nc.sync.dma_start(out=d_out[b * P:b * P + rows, :],
                                  in_=osb[:rows, :])

        pers.release()
        dpool.release()

    nc.compile()
    return nc


# ----------------------------------------------------------------------------
# driver
# ----------------------------------------------------------------------------


def run(inputs, cfg, trace=False):
    from concourse import bass_utils

    NC, N, DOUT = cfg["NC"], cfg["N"], cfg["DOUT"]
    in_maps, meta = preprocess(inputs, cfg)
    nc = build_program(meta, cfg)
    res = bass_utils.run_bass_kernel_spmd(
        nc, in_maps, core_ids=list(range(NC)), trace=trace,
    )
    out = np.concatenate([res.results[c]["out"] for c in range(NC)], axis=0)
    return np.ascontiguousarray(out[:N]).astype(np.float32), res


# revision 9
# speedup vs baseline: 1.4470x; 1.4470x over previous
"""GCN (4-layer message-passing) Trainium2 kernel, 8-core SPMD.

Math (matches PyG GCNConv with self-loops, per reference):
    deg[d]  = in-degree over (edges + self-loops)
    dinv    = deg^-1/2
    h0      = x @ W_emb + b_emb
    layer i: h <- tanh( dinv[d] * sum_{e: dst=d} dinv[src_e] * (h @ W_i)[src_e] + b_i )
    out     = h @ W_out + b_out

Distribution: nodes sharded across 8 cores (dst-sharded edges). Per layer:
  1. transform own shard:  hWd = dinv * (h @ W)   (PE matmul + ACT scale/cast bf16)
  2. AllGather hWd across cores (bf16, HBM collective)
  3. dma_gather (SWDGE) each in-edge's source row, sorted by dst
  4. segment-sum via PE matmuls against host-built one-hot selection tiles
     (exact in bf16), accumulated in PSUM; bias added via rank-1 matmul
  5. tanh with per-partition dinv scale on ACT; PE-transpose back to h^T

The embedding layer is folded into layer 1's weights host-side
(x @ (W_emb@W_1) + (b_emb@W_1)), so it never runs on device.

int16 gather indices cap the addressable rows at 32768, so the AllGather'd
feature table is gathered through two views (rows [0,32768) and [32768,...)),
with each block's edges grouped lo-half-first.
"""

import math

import ml_dtypes
import numpy as np

BF16 = ml_dtypes.bfloat16
P = 128

CFG_FULL = dict(N=50000, E=800000, DIN=128, DH=256, DOUT=64, L=4, NC=8)

# blocks of 128 dst nodes handled per gather/S-matrix chunk
CHUNK_BLOCKS = 2


def kernel(**inputs) -> np.ndarray:
    out, _ = run(inputs, CFG_FULL)
    return out


# ----------------------------------------------------------------------------
# host-side preprocessing
# ----------------------------------------------------------------------------


def _ceil_div(a, b):
    return (a + b - 1) // b


def preprocess(inputs, cfg):
    N, E, DIN, DH, DOUT, L, NC = (
        cfg["N"], cfg["E"], cfg["DIN"], cfg["DH"], cfg["DOUT"], cfg["L"], cfg["NC"],
    )
    x = np.asarray(inputs["x"], np.float32)
    ei = np.asarray(inputs["edge_index"]).astype(np.int64)
    W_emb = np.asarray(inputs["W_emb"], np.float32)
    b_emb = np.asarray(inputs["b_emb"], np.float32)
    W_conv = np.asarray(inputs["W_conv"], np.float32)
    b_conv = np.asarray(inputs["b_conv"], np.float32)
    W_out = np.asarray(inputs["W_out"], np.float32)
    b_out = np.asarray(inputs["b_out"], np.float32)

    loop = np.arange(N, dtype=np.int64)
    src = np.concatenate([ei[0], loop])
    dst = np.concatenate([ei[1], loop])
    deg = np.bincount(dst, minlength=N).astype(np.float32)
    dinv = (1.0 / np.sqrt(np.maximum(deg, 1.0))).astype(np.float32)
    sqdeg = np.sqrt(np.maximum(deg, 1.0)).astype(np.float32)

    NPs = _ceil_div(N, NC)          # real nodes per shard
    NB = _ceil_div(NPs, P)          # dst blocks per core
    NPP = NB * P                    # padded nodes per shard
    AGR = NC * NPP                  # allgather rows
    HALF = 32768
    has_hi = AGR > HALF

    agrow = (src // NPs) * NPP + (src % NPs)
    core_of = dst // NPs
    d_loc = dst - core_of * NPs
    blk = d_loc // P
    col = d_loc % P
    half = (agrow >= HALF).astype(np.int64)

    # per-core edge partitions, sorted by (block, half, dst, src)
    cores = []
    nseg = np.zeros((NC, NB, 2), np.int64)
    for c in range(NC):
        m = core_of == c
        a_blk, a_half, a_col, a_sag = blk[m], half[m], col[m], agrow[m]
        order = np.lexsort((a_sag, a_col, a_half, a_blk))
        a_blk, a_half, a_col, a_sag = (
            a_blk[order], a_half[order], a_col[order], a_sag[order],
        )
        cnt = np.bincount(a_blk * 2 + a_half, minlength=NB * 2).reshape(NB, 2)
        nseg[c] = cnt
        cores.append((a_blk, a_half, a_col, a_sag))

    nmax = nseg.max(axis=0)                      # [NB, 2]
    T = np.maximum(_ceil_div(nmax, P), 0)        # tiles per (block, half)
    T[:, 0] = np.maximum(T[:, 0], 1)             # every block has >= 1 lo tile slot
    if not has_hi:
        T[:, 1] = 0

    # chunk layout (identical across cores)
    chunks = []
    gidx_col = 0
    tile_ctr = 0
    for g0 in range(0, NB, CHUNK_BLOCKS):
        blocks = list(range(g0, min(g0 + CHUNK_BLOCKS, NB)))
        tlo = int(T[blocks, 0].sum())
        thi = int(T[blocks, 1].sum())
        lo_cols = (gidx_col, gidx_col + tlo * P // 16)
        gidx_col = lo_cols[1]
        hi_cols = (gidx_col, gidx_col + thi * P // 16)
        gidx_col = hi_cols[1]
        # tile positions: lo tiles of blocks in order, then hi tiles
        lo_base, hi_base = {}, {}
        t = 0
        for b in blocks:
            lo_base[b] = t
            t += int(T[b, 0])
        t = 0
        for b in blocks:
            hi_base[b] = t
            t += int(T[b, 1])
        smat_tiles = (tile_ctr, tile_ctr + tlo + thi)
        tile_ctr = smat_tiles[1]
        chunks.append(dict(
            blocks=blocks, tlo=tlo, thi=thi, lo_cols=lo_cols, hi_cols=hi_cols,
            lo_base=lo_base, hi_base=hi_base, smat_tiles=smat_tiles,
        ))
    GC = gidx_col
    TT = tile_ctr

    meta = dict(
        NPs=NPs, NB=NB, NPP=NPP, AGR=AGR, HALF=HALF, has_hi=has_hi,
        T=T, chunks=chunks, GC=GC, TT=TT,
    )

    # shared weights
    W1x = (W_emb @ W_conv[0]).astype(BF16)                    # [DIN, DH]
    bemb1 = (b_emb @ W_conv[0]).reshape(1, DH).astype(BF16)
    Wc = W_conv[1:].reshape((L - 1) * DH, DH).astype(BF16) if L > 1 else \
        np.zeros((0, DH), BF16)
    bc = b_conv.reshape(1, L * DH).astype(BF16)
    Wo = W_out.astype(BF16)                                    # [DH, DOUT]
    bo = b_out.reshape(1, DOUT).astype(BF16)

    in_maps = []
    for c in range(NC):
        a_blk, a_half, a_col, a_sag = cores[c]
        n0 = c * NPs
        n1 = min(n0 + NPs, N)
        nreal = n1 - n0

        # per-edge destination slots in the (chunk-ordered) tile stream
        # segment offsets within each (blk, half) run
        seg_id = a_blk * 2 + a_half
        seg_start = np.zeros(NB * 2, np.int64)
        cnts = np.bincount(seg_id, minlength=NB * 2)
        seg_start[1:] = np.cumsum(cnts)[:-1]
        epos = np.arange(len(seg_id)) - seg_start[seg_id]     # offset in segment

        # global tile index for each edge
        tile_of_seg = np.zeros(NB * 2, np.int64)
        for ch in chunks:
            for b in ch["blocks"]:
                tile_of_seg[b * 2] = ch["smat_tiles"][0] + ch["lo_base"][b]
                tile_of_seg[b * 2 + 1] = (
                    ch["smat_tiles"][0] + ch["tlo"] + ch["hi_base"][b]
                )
        e_tile = tile_of_seg[seg_id] + epos // P
        e_row = epos % P

        # selection matrices [128, TT*128] bf16, one-hot (exact)
        smat = np.zeros((P, TT * P), BF16)
        smat[e_row, e_tile * P + a_col] = BF16(1.0)

        # gather indices, wrapped layout [128, GC] int16
        gidx = np.zeros((16, GC), np.int16)
        for ch in chunks:
            for h, colrange, base_map, tcount in (
                (0, ch["lo_cols"], ch["lo_base"], ch["tlo"]),
                (1, ch["hi_cols"], ch["hi_base"], ch["thi"]),
            ):
                if tcount == 0:
                    continue
                vals = np.zeros(tcount * P, np.int64)
                for b in ch["blocks"]:
                    m = (a_blk == b) & (a_half == h)
                    v = a_sag[m] - (HALF if h else 0)
                    off = base_map[b] * P
                    vals[off:off + len(v)] = v
                c0, c1 = colrange
                gidx[:, c0:c1] = vals.reshape(c1 - c0, 16).T
        gidx = np.tile(gidx, (8, 1)).astype(np.int16)          # replicate x8

        # dinv [128, NB] fp32 ; sqdeg [1, NPP] bf16
        dl = np.ones(NPP, np.float32)
        dl[:nreal] = dinv[n0:n1]
        dinvp = dl.reshape(NB, P).T.copy()                     # [P, NB]
        sq = np.ones(NPP, np.float32)
        sq[:nreal] = sqdeg[n0:n1]
        sqdegp = sq.reshape(1, NPP).astype(BF16)

        xT = np.zeros((DIN, NPP), BF16)
        xT[:, :nreal] = x[n0:n1].T

        in_maps.append(dict(
            xT=np.ascontiguousarray(xT),
            gidx=np.ascontiguousarray(gidx),
            smat=np.ascontiguousarray(smat),
            dinvp=np.ascontiguousarray(dinvp),
            sqdegp=np.ascontiguousarray(sqdegp),
            w1x=W1x, bemb1=bemb1, wc=Wc, bc=bc, wo=Wo, bo=bo,
        ))

    return in_maps, meta


# ----------------------------------------------------------------------------
# device program
# ----------------------------------------------------------------------------


def build_program(meta, cfg):
    import concourse.bacc as bacc
    import concourse.mybir as mybir
    import concourse.tile as tile
    from concourse.masks import make_identity

    N, DIN, DH, DOUT, L, NC = (
        cfg["N"], cfg["DIN"], cfg["DH"], cfg["DOUT"], cfg["L"], cfg["NC"],
    )
    NPs, NB, NPP, AGR, HALF = (
        meta["NPs"], meta["NB"], meta["NPP"], meta["AGR"], meta["HALF"],
    )
    T, chunks, GC, TT = meta["T"], meta["chunks"], meta["GC"], meta["TT"]
    has_hi = meta["has_hi"]
    assert DIN == P

    f32 = mybir.dt.float32
    bf16 = mybir.dt.bfloat16
    i16 = mybir.dt.int16
    TANH = mybir.ActivationFunctionType.Tanh
    COPY = mybir.ActivationFunctionType.Copy

    nc = bacc.Bacc("TRN2", target_bir_lowering=False, debug=False, num_devices=NC,
                   num_swdge_queues=4)

    # I/O
    d_xT = nc.dram_tensor("xT", [DIN, NPP], bf16, kind="ExternalInput")
    d_gidx = nc.dram_tensor("gidx", [P, GC], i16, kind="ExternalInput")
    d_smat = nc.dram_tensor("smat", [P, TT * P], bf16, kind="ExternalInput")
    d_dinv = nc.dram_tensor("dinvp", [P, NB], f32, kind="ExternalInput")
    d_sqdeg = nc.dram_tensor("sqdegp", [1, NPP], bf16, kind="ExternalInput")
    d_w1x = nc.dram_tensor("w1x", [DIN, DH], bf16, kind="ExternalInput")
    d_bemb1 = nc.dram_tensor("bemb1", [1, DH], bf16, kind="ExternalInput")
    d_wc = nc.dram_tensor("wc", [(L - 1) * DH, DH], bf16, kind="ExternalInput")
    d_bc = nc.dram_tensor("bc", [1, L * DH], bf16, kind="ExternalInput")
    d_wo = nc.dram_tensor("wo", [DH, DOUT], bf16, kind="ExternalInput")
    d_bo = nc.dram_tensor("bo", [1, DOUT], bf16, kind="ExternalInput")
    d_out = nc.dram_tensor("out", [NPs, DOUT], f32, kind="ExternalOutput")

    with tile.TileContext(nc) as tc:
        pers = tc.alloc_tile_pool(name="pers", bufs=1)
        dpool = tc.alloc_tile_pool(name="dpers", bufs=1, space="DRAM")

        # internal DRAM for the collective
        agin = dpool.tile([NPP, DH], bf16, name="agin", tag="agin")
        agouts = [
            dpool.tile([AGR, DH], bf16, name=f"agout{i}", tag=f"agout{i}",
                       addr_space="Shared")
            for i in range(L)
        ]

        def stile(shape, dtype, name):
            return pers.tile(shape, dtype, name=name, tag=name)

        # persistent SBUF state
        xT = stile([DIN, NPP], bf16, "xT_sb")
        hT0 = stile([P, NPP], bf16, "hT0_sb")
        hT1 = stile([P, NPP], bf16, "hT1_sb")
        gidx = stile([P, GC], i16, "gidx_sb")
        dinv = stile([P, NB], f32, "dinv_sb")
        sqd = stile([1, NPP], bf16, "sqdeg_sb")
        w1x = stile([DIN, DH], bf16, "w1x_sb")
        bemb1 = stile([1, DH], bf16, "bemb1_sb")
        wc = stile([P, 2 * (L - 1) * DH], bf16, "wc_sb")
        bc = stile([1, L * DH], bf16, "bc_sb")
        wo = stile([P, 2 * DOUT], bf16, "wo_sb")
        bo = stile([1, DOUT], bf16, "bo_sb")
        ident = stile([P, P], bf16, "ident_sb")
        ones = stile([1, P], bf16, "ones_sb")

        nc.sync.dma_start(out=xT[:], in_=d_xT[:])
        nc.sync.dma_start(out=gidx[:], in_=d_gidx[:])
        nc.sync.dma_start(out=dinv[:], in_=d_dinv[:])
        nc.sync.dma_start(out=sqd[:], in_=d_sqdeg[:])
        nc.sync.dma_start(out=w1x[:], in_=d_w1x[:])
        nc.sync.dma_start(out=bemb1[:], in_=d_bemb1[:])
        for i in range(L - 1):
            for k in range(2):
                nc.sync.dma_start(
                    out=wc[:, (2 * i + k) * DH:(2 * i + k + 1) * DH],
                    in_=d_wc[i * DH + k * P:i * DH + (k + 1) * P, :],
                )
        nc.sync.dma_start(out=bc[:], in_=d_bc[:])
        for k in range(2):
            nc.sync.dma_start(
                out=wo[:, k * DOUT:(k + 1) * DOUT],
                in_=d_wo[k * P:(k + 1) * P, :],
            )
        nc.sync.dma_start(out=bo[:], in_=d_bo[:])
        make_identity(nc, ident[:])
        nc.gpsimd.memset(ones[:], 1.0)

        with tc.tile_pool(name="work", bufs=2) as wp, \
                tc.tile_pool(name="psum", bufs=2, space="PSUM") as pp:

            for i in range(L):
                # ---- transform: hWd = dinv * (h @ W_i), cast bf16 -> agin
                for b in range(NB):
                    pt = pp.tile([P, DH], f32, tag="pt")
                    bs = slice(b * P, (b + 1) * P)
                    if i == 0:
                        nc.tensor.matmul(
                            out=pt[:], lhsT=xT[:, bs], rhs=w1x[:],
                            start=True, stop=False,
                        )
                        nc.tensor.matmul(
                            out=pt[:], lhsT=ones[:, :], rhs=bemb1[:],
                            start=False, stop=True,
                        )
                    else:
                        j = i - 1
                        nc.tensor.matmul(
                            out=pt[:], lhsT=hT0[:, bs],
                            rhs=wc[:, (2 * j) * DH:(2 * j + 1) * DH],
                            start=True, stop=False,
                        )
                        nc.tensor.matmul(
                            out=pt[:], lhsT=hT1[:, bs],
                            rhs=wc[:, (2 * j + 1) * DH:(2 * j + 2) * DH],
                            start=False, stop=True,
                        )
                    hwd = wp.tile([P, DH], bf16, tag="hwd")
                    nc.scalar.activation(
                        out=hwd[:], in_=pt[:], func=COPY,
                        scale=dinv[:, b:b + 1],
                    )
                    nc.sync.dma_start(out=agin[bs, :], in_=hwd[:])

                # ---- allgather transformed features
                nc.gpsimd.collective_compute(
                    "AllGather",
                    mybir.AluOpType.bypass,
                    replica_groups=[list(range(NC))],
                    ins=[agin[:]],
                    outs=[agouts[i][:]],
                )

                # ---- gather + segment-sum + tanh + transpose
                for ci, ch in enumerate(chunks):
                    tlo, thi = ch["tlo"], ch["thi"]
                    msg_lo = wp.tile([P, tlo * DH], bf16, tag="msglo", bufs=3)
                    nc.gpsimd.dma_gather(
                        out_ap=msg_lo[:].rearrange("p (t e) -> p t e", e=DH),
                        in_ap=agouts[i][:min(HALF, AGR), :],
                        idxs_ap=gidx[:, ch["lo_cols"][0]:ch["lo_cols"][1]],
                        num_idxs=tlo * P,
                        num_idxs_reg=tlo * P,
                        elem_size=DH,
                        single_packet=False,
                        queue_num=(2 * ci) % 4,
                    )
                    if thi > 0:
                        msg_hi = wp.tile([P, thi * DH], bf16, tag="msghi", bufs=3)
                        nc.gpsimd.dma_gather(
                            out_ap=msg_hi[:].rearrange("p (t e) -> p t e", e=DH),
                            in_ap=agouts[i][HALF:AGR, :],
                            idxs_ap=gidx[:, ch["hi_cols"][0]:ch["hi_cols"][1]],
                            num_idxs=thi * P,
                            num_idxs_reg=thi * P,
                            elem_size=DH,
                            single_packet=False,
                            queue_num=(2 * ci + 1) % 4,
                        )
                    smt = wp.tile([P, (tlo + thi) * P], bf16, tag="smat", bufs=3)
                    t0 = ch["smat_tiles"][0]
                    nc.sync.dma_start(
                        out=smt[:], in_=d_smat[:, t0 * P:(t0 + tlo + thi) * P],
                    )

                    for b in ch["blocks"]:
                        pa = pp.tile([P, DH], f32, tag="pa", bufs=4)
                        nmm = int(T[b, 0]) + int(T[b, 1])
                        j = 0
                        for t in range(int(T[b, 0])):
                            s_pos = ch["lo_base"][b] + t
                            nc.tensor.matmul(
                                out=pa[:],
                                lhsT=smt[:, s_pos * P:(s_pos + 1) * P],
                                rhs=msg_lo[:, s_pos * DH:(s_pos + 1) * DH],
                                start=(j == 0), stop=False,
                            )
                            j += 1
                        for t in range(int(T[b, 1])):
                            s_pos = ch["hi_base"][b] + t
                            nc.tensor.matmul(
                                out=pa[:],
                                lhsT=smt[:, (tlo + s_pos) * P:(tlo + s_pos + 1) * P],
                                rhs=msg_hi[:, s_pos * DH:(s_pos + 1) * DH],
                                start=(j == 0), stop=False,
                            )
                            j += 1
                        assert j == nmm
                        bs = slice(b * P, (b + 1) * P)
                        # + sqdeg[d] * b_i  (cancels the dinv scale below)
                        nc.tensor.matmul(
                            out=pa[:], lhsT=sqd[:, bs],
                            rhs=bc[:, i * DH:(i + 1) * DH],
                            start=False, stop=True,
                        )
                        hnew = wp.tile([P, DH], bf16, tag="hnew")
                        nc.scalar.activation(
                            out=hnew[:], in_=pa[:], func=TANH,
                            scale=dinv[:, b:b + 1],
                        )
                        for k, hT in enumerate((hT0, hT1)):
                            ptr = pp.tile([P, P], bf16, tag="ptr")
                            nc.tensor.transpose(
                                out=ptr[:], in_=hnew[:, k * P:(k + 1) * P],
                                identity=ident[:],
                            )
                            nc.vector.tensor_copy(out=hT[:, bs], in_=ptr[:])

            # ---- output projection
            for b in range(NB):
                po = pp.tile([P, DOUT], f32, tag="pt")
                bs = slice(b * P, (b + 1) * P)
                nc.tensor.matmul(out=po[:], lhsT=hT0[:, bs], rhs=wo[:, :DOUT],
                                 start=True, stop=False)
                nc.tensor.matmul(out=po[:], lhsT=hT1[:, bs],
                                 rhs=wo[:, DOUT:2 * DOUT],
                                 start=False, stop=False)
                nc.tensor.matmul(out=po[:], lhsT=ones[:, :], rhs=bo[:],
                                 start=False, stop=True)
                osb = wp.tile([P, DOUT], f32, tag="osb")
                nc.scalar.activation(out=osb[:], in_=po[:], func=COPY)
                rows = min(P, NPs - b * P)
                nc.sync.dma_start(out=d_out[b * P:b * P + rows, :],
                                  in_=osb[:rows, :])

        pers.release()
        dpool.release()

    nc.compile()
    return nc


# ----------------------------------------------------------------------------
# driver
# ----------------------------------------------------------------------------


def run(inputs, cfg, trace=False):
    from concourse import bass_utils

    NC, N, DOUT = cfg["NC"], cfg["N"], cfg["DOUT"]
    in_maps, meta = preprocess(inputs, cfg)
    nc = build_program(meta, cfg)
    res = bass_utils.run_bass_kernel_spmd(
        nc, in_maps, core_ids=list(range(NC)), trace=trace,
    )
    out = np.concatenate([res.results[c]["out"] for c in range(NC)], axis=0)
    return np.ascontiguousarray(out[:N]).astype(np.float32), res
